# revision 8
# baseline (speedup 1.0000x reference)
"""Trainium2 Bass kernel for a single-head attention variant.

Reference math (per batch element b):
    q = x @ Wq.T            [S, H]
    k = x @ Wk.T            [S, H]
    v = q                   (source bug: v computed with q_net)
    s = q @ k.T / sqrt(S)   [Sq, Sk]
    a = softmax(s, axis=q)  (normalizes over the QUERY axis, per key column)
    out = a @ v             [S, H]

Key identity used: since softmax normalizes over q (per column k), and k is
the contraction index of the final matmul,
    out[q, h] = sum_k exp(s[q,k]) * (v[k,h] / Z[k]),   Z[k] = sum_q exp(s[q,k])
so the big exp matrix never needs normalizing; the 1/Z folds into V rows.
The score magnitudes are bounded (|s| <~ 1), so no max-subtraction is needed.

Sharding: pure data-parallel — batch 8 over the 8 NeuronCores, no collectives.

Per-core dataflow (all matmuls bf16, f32 PSUM accumulate):
  phase 1: QT[64,S], KT[64,S] = W{q,k}^T.T @ X^T      (X^T fed from host)
  phase 2: V[k,h] chunks = PE-transpose of QT
  main loop over 32 key chunks kc (128 keys each):
    ST[kc]   = KT[:,kc].T @ QT          [128, S] scores, via PSUM
    PT[kc]   = exp(ST/64)  (ScalarE, fused scale, fused row-sum -> Z[kc])
    V'[kc]   = V[kc] * (1/Z[kc])
    out^T   += V'[kc].T @ PT[kc]        accumulated in PSUM across all kc,
                                        q-halves packed at partitions 0-63/64-127
"""

import numpy as np
import ml_dtypes

B, SEQ, EMBED, HEAD = 8, 4096, 768, 64
N_CORES = 8


def build_graph(seq=SEQ):
    from contextlib import ExitStack

    import concourse.bass as bass
    import concourse.mybir as mybir
    import concourse.tile as tile
    from concourse import bacc
    from concourse.masks import make_identity

    f32 = mybir.dt.float32
    bf16 = mybir.dt.bfloat16
    AF = mybir.ActivationFunctionType

    E, H = EMBED, HEAD
    EC = E // 128          # 6 embed chunks
    KC = seq // 128        # key chunks
    QB = seq // 512        # 512-wide q blocks
    HQ = seq // 2          # half of q range (packed at partitions 64-127)
    BW = min(512, HQ)      # matmul block width
    HB = HQ // BW          # q blocks per half
    scale = 1.0 / float(np.sqrt(np.float32(seq)))

    nc = bacc.Bacc("TRN2", target_bir_lowering=False, debug=False)

    xt_d = nc.dram_tensor("xt", [E, seq], bf16, kind="ExternalInput")
    wq_d = nc.dram_tensor("wq", [E, H], bf16, kind="ExternalInput")
    wk_d = nc.dram_tensor("wk", [E, H], bf16, kind="ExternalInput")
    # out^T packed: partitions 0-63 = q[0:HQ], partitions 64-127 = q[HQ:2HQ]
    out_d = nc.dram_tensor("out", [128, HQ], f32, kind="ExternalOutput")

    with tile.TileContext(nc) as tc, ExitStack() as ctx:
        const_p = ctx.enter_context(tc.tile_pool(name="const", bufs=1))
        ident = const_p.tile([64, 64], bf16)
        make_identity(nc, ident[:])

        # weights: [128, proj, ec, H] so lhsT slices are [128, 64]
        w_sb = const_p.tile([128, 2, EC, H], bf16)
        nc.sync.dma_start(
            out=w_sb[:, 0], in_=wq_d.ap().rearrange("(c p) h -> p c h", p=128)
        )
        nc.sync.dma_start(
            out=w_sb[:, 1], in_=wk_d.ap().rearrange("(c p) h -> p c h", p=128)
        )

        qt_sb = const_p.tile([64, seq], bf16)
        kt_sb = const_p.tile([64, seq], bf16)
        v_sb = const_p.tile([128, KC, H], bf16)

        # ---- phase 1: projections QT/KT = W.T @ X^T ----
        with tc.tile_pool(name="xt", bufs=1) as xt_p, tc.tile_pool(
            name="proj_ps", bufs=4, space="PSUM"
        ) as proj_ps:
            xt_tiles = []
            for ec in range(EC):
                t = xt_p.tile([128, seq], bf16, tag=f"xt{ec}")
                nc.sync.dma_start(out=t[:], in_=xt_d.ap()[ec * 128 : (ec + 1) * 128, :])
                xt_tiles.append(t)
            for proj, dst in ((0, qt_sb), (1, kt_sb)):
                for qb in range(QB):
                    ps = proj_ps.tile([64, 512], f32, tag="proj")
                    for ec in range(EC):
                        nc.tensor.matmul(
                            ps[:],
                            lhsT=w_sb[:, proj, ec, :],
                            rhs=xt_tiles[ec][:, qb * 512 : (qb + 1) * 512],
                            start=(ec == 0),
                            stop=(ec == EC - 1),
                        )
                    if proj == 0:
                        nc.vector.tensor_copy(dst[:, qb * 512 : (qb + 1) * 512], ps[:])
                    else:
                        nc.scalar.copy(dst[:, qb * 512 : (qb + 1) * 512], ps[:])

            # ---- phase 2: V chunks = transpose(QT) ----
            with tc.tile_pool(name="v_ps", bufs=4, space="PSUM") as v_ps:
                for kc in range(KC):
                    vt = v_ps.tile([128, H], bf16, tag="vt")
                    nc.tensor.transpose(
                        vt[:], qt_sb[:, kc * 128 : (kc + 1) * 128], ident[:]
                    )
                    nc.vector.tensor_copy(v_sb[:, kc, :], vt[:])

        # ---- main loop ----
        with tc.tile_pool(name="s_ps", bufs=1, space="PSUM") as s_ps_p, tc.tile_pool(
            name="o_ps", bufs=1, space="PSUM"
        ) as o_ps_p, tc.tile_pool(name="pt", bufs=3) as pt_p, tc.tile_pool(
            name="zz", bufs=6
        ) as z_p, tc.tile_pool(name="v2", bufs=3) as v2_p:
            out_ps = o_ps_p.tile([128, HQ], f32)
            # Zero-init every out bank with a full-partition start=True matmul
            # so the packed q-halves (partitions 0-63 / 64-127) can then both
            # accumulate with start=False — avoids interleaved-group hazards
            # on the bank-wide has_written clear.
            zw = const_p.tile([128, max(BW, 128)], bf16)
            nc.vector.memset(zw[:], 0.0)
            for j in range(HB):
                nc.tensor.matmul(
                    out_ps[:, j * BW : (j + 1) * BW],
                    lhsT=zw[:, 0:128],
                    rhs=zw[:, 0:BW],
                    start=True,
                    stop=False,
                )
            for kc in range(KC):
                kslice = kt_sb[:, kc * 128 : (kc + 1) * 128]
                zacc = z_p.tile([128, 2], f32, tag="zacc")
                pts = []
                for half in range(2):
                    s_ps = s_ps_p.tile([128, HQ], f32, tag="s")
                    for j in range(HB):
                        q0 = half * HQ + j * BW
                        nc.tensor.matmul(
                            s_ps[:, j * BW : (j + 1) * BW],
                            lhsT=kslice,
                            rhs=qt_sb[:, q0 : q0 + BW],
                            start=True,
                            stop=True,
                        )
                    pt = pt_p.tile([128, HQ], bf16, tag="pt")
                    pts.append(pt)
                    nc.scalar.activation(
                        pt[:],
                        s_ps[:],
                        AF.Exp,
                        scale=scale,
                        accum_out=zacc[:, half : half + 1],
                    )
                z = z_p.tile([128, 1], f32, tag="z")
                nc.vector.tensor_add(z[:], zacc[:, 0:1], zacc[:, 1:2])
                rz = z_p.tile([128, 1], f32, tag="rz")
                nc.vector.reciprocal(rz[:], z[:])
                v2 = v2_p.tile([128, H], bf16, tag="v2")
                nc.vector.tensor_scalar_mul(v2[:], v_sb[:, kc, :], rz[:])
                for half in range(2):
                    for j in range(HB):
                        nc.tensor.matmul(
                            out_ps[
                                half * 64 : (half + 1) * 64, j * BW : (j + 1) * BW
                            ],
                            lhsT=v2[:],
                            rhs=pts[half][:, j * BW : (j + 1) * BW],
                            start=False,
                            stop=False,
                        )
            # close every accumulation group with a full-partition zero-add
            for j in range(HB):
                nc.tensor.matmul(
                    out_ps[:, j * BW : (j + 1) * BW],
                    lhsT=zw[:, 0:128],
                    rhs=zw[:, 0:BW],
                    start=False,
                    stop=True,
                )
            out_sb = const_p.tile([128, HQ], f32)
            nc.vector.tensor_copy(out_sb[:], out_ps[:])
            nc.sync.dma_start(out=out_d.ap(), in_=out_sb[:])

    nc.compile()
    return nc


_NC_CACHE = {}


def _get_nc(seq=SEQ):
    if seq not in _NC_CACHE:
        _NC_CACHE[seq] = build_graph(seq)
    return _NC_CACHE[seq]


def make_in_maps(input_ids, Wq, Wk):
    bf = ml_dtypes.bfloat16
    wq_t = np.ascontiguousarray(Wq.T).astype(bf)  # [E, H]
    wk_t = np.ascontiguousarray(Wk.T).astype(bf)
    in_maps = []
    for b in range(input_ids.shape[0]):
        xt = np.ascontiguousarray(input_ids[b].T).astype(bf)  # [E, S]
        in_maps.append({"xt": xt, "wq": wq_t, "wk": wk_t})
    return in_maps


def unpack_out(packed, seq=SEQ):
    """[128, seq/2] packed out^T -> [seq, H] out."""
    ot = np.concatenate([packed[:64], packed[64:]], axis=1)  # [64, seq]
    return np.ascontiguousarray(ot.T)


def run(input_ids, Wq, Wk, trace=False, **kwargs):
    from concourse.bass_utils import run_bass_kernel_spmd

    nc = _get_nc(input_ids.shape[1])
    in_maps = make_in_maps(input_ids, Wq, Wk)
    res = run_bass_kernel_spmd(
        nc, in_maps, core_ids=list(range(len(in_maps))), trace=trace, **kwargs
    )
    out = np.stack(
        [
            unpack_out(res.results[b]["out"], input_ids.shape[1])
            for b in range(len(in_maps))
        ]
    ).astype(np.float32)
    return out, res


def kernel(input_ids, Wq, Wk, Wv):
    out, _ = run(
        np.asarray(input_ids), np.asarray(Wq), np.asarray(Wk)
    )
    return out


# revision 11
# speedup vs baseline: 1.4497x; 1.4497x over previous
"""Trainium2 Bass kernel for a single-head attention variant.

Reference math (per batch element b):
    q = x @ Wq.T            [S, H]
    k = x @ Wk.T            [S, H]
    v = q                   (source bug: v computed with q_net)
    s = q @ k.T / sqrt(S)   [Sq, Sk]
    a = softmax(s, axis=q)  (normalizes over the QUERY axis, per key column)
    out = a @ v             [S, H]

Key identity used: since softmax normalizes over q (per column k), and k is
the contraction index of the final matmul,
    out[q, h] = sum_k exp(s[q,k]) * (v[k,h] / Z[k]),   Z[k] = sum_q exp(s[q,k])
so the big exp matrix never needs normalizing; the 1/Z folds into V rows.
The score magnitudes are bounded (|s| <~ 1), so no max-subtraction is needed.

Sharding: pure data-parallel — batch 8 over the 8 NeuronCores, no collectives.

Per-core dataflow (all matmuls bf16, f32 PSUM accumulate):
  phase 1: [QT;QT] and [KT;KT] = W2.T @ X^T   (X^T and the column-duplicated
           W2 = [W^T|W^T] fed from host; duplication puts QT/KT on both
           partition halves so the score matmuls can be row-tiled)
  phase 2: V[k,h] chunks = PE-transpose of QT
  main loop over key-chunk pairs (2 x 128 keys, row-tiled in the PE array):
    ST[kc]   = KT[:,kc].T @ QT            [128, S] scores, 1024-wide windows
    PT[kc]   = exp(ST/64)  (ScalarE, fused scale; row-sums Z via fused
               accum_out on half the windows, DVE reduce on the rest)
    V'[kc]   = V[kc] * (1/Z[kc])
    out^T   += V'[kc].T @ PT[kc]          accumulated in PSUM across all kc,
                                          q-halves packed at partitions
                                          0-63 / 64-127 (col-tiled matmuls)
"""

import numpy as np
import ml_dtypes

B, SEQ, EMBED, HEAD = 8, 4096, 768, 64
N_CORES = 8


def build_graph(seq=SEQ):
    from contextlib import ExitStack

    import concourse.mybir as mybir
    import concourse.tile as tile
    from concourse import bacc
    from concourse.masks import make_identity

    f32 = mybir.dt.float32
    bf16 = mybir.dt.bfloat16
    AF = mybir.ActivationFunctionType

    E, H = EMBED, HEAD
    EC = E // 128          # 6 embed chunks
    KC = seq // 128        # key chunks
    QB = seq // 512        # 512-wide q blocks
    HQ = seq // 2          # half of q range (packed at partitions 64-127)
    BW = min(512, HQ)      # matmul block width
    HB = HQ // BW          # q blocks per half
    WW = min(1024, seq)    # exp window width
    NW = seq // WW         # windows per key chunk
    NB = WW // BW          # matmul blocks per window
    scale = 1.0 / float(np.sqrt(np.float32(seq)))

    nc = bacc.Bacc("TRN2", target_bir_lowering=False, debug=False)

    xt_d = nc.dram_tensor("xt", [E, seq], bf16, kind="ExternalInput")
    # column-duplicated transposed projection weights: [E, 128] = [W^T | W^T]
    wq_d = nc.dram_tensor("wq2", [E, 128], bf16, kind="ExternalInput")
    wk_d = nc.dram_tensor("wk2", [E, 128], bf16, kind="ExternalInput")
    # out^T packed: partitions 0-63 = q[0:HQ], partitions 64-127 = q[HQ:2HQ]
    out_d = nc.dram_tensor("out", [128, HQ], f32, kind="ExternalOutput")

    with tile.TileContext(nc) as tc, ExitStack() as ctx:
        const_p = ctx.enter_context(tc.tile_pool(name="const", bufs=1))
        ident = const_p.tile([64, 64], bf16)
        make_identity(nc, ident[:])

        w_sb = const_p.tile([128, 2, EC, 128], bf16)
        nc.sync.dma_start(
            out=w_sb[:, 0], in_=wq_d.ap().rearrange("(c p) h -> p c h", p=128)
        )
        nc.sync.dma_start(
            out=w_sb[:, 1], in_=wk_d.ap().rearrange("(c p) h -> p c h", p=128)
        )

        qq_sb = const_p.tile([128, seq], bf16)  # [QT; QT] on both halves
        kk_sb = const_p.tile([128, seq], bf16)  # [KT; KT]
        v_sb = const_p.tile([128, KC, H], bf16)

        # ---- phase 1: projections, ec-outer so DMA overlaps compute ----
        with tc.tile_pool(name="xt", bufs=1) as xt_p, tc.tile_pool(
            name="proj_ps", bufs=1, space="PSUM"
        ) as proj_ps:
            xt_tiles = []
            for ec in range(EC):
                t = xt_p.tile([128, seq], bf16, tag=f"xt{ec}")
                nc.sync.dma_start(out=t[:], in_=xt_d.ap()[ec * 128 : (ec + 1) * 128, :])
                xt_tiles.append(t)
            for proj, dst in ((0, qq_sb), (1, kk_sb)):
                big = proj_ps.tile([128, seq], f32, tag="proj")
                for ec in range(EC):
                    for qb in range(QB):
                        nc.tensor.matmul(
                            big[:, qb * 512 : (qb + 1) * 512],
                            lhsT=w_sb[:, proj, ec, :],
                            rhs=xt_tiles[ec][:, qb * 512 : (qb + 1) * 512],
                            start=(ec == 0),
                            stop=(ec == EC - 1),
                        )
                for qb in range(QB):
                    eng = nc.vector if qb % 2 == 0 else nc.scalar
                    if qb % 2 == 0:
                        nc.vector.tensor_copy(
                            dst[:, qb * 512 : (qb + 1) * 512],
                            big[:, qb * 512 : (qb + 1) * 512],
                        )
                    else:
                        nc.scalar.copy(
                            dst[:, qb * 512 : (qb + 1) * 512],
                            big[:, qb * 512 : (qb + 1) * 512],
                        )

        # ---- phase 2: V chunks = transpose(QT) ----
        with tc.tile_pool(name="v_ps", bufs=4, space="PSUM") as v_ps:
            for kc in range(KC):
                vt = v_ps.tile([128, H], bf16, tag="vt")
                nc.tensor.transpose(
                    vt[:], qq_sb[0:64, kc * 128 : (kc + 1) * 128], ident[:]
                )
                nc.vector.tensor_copy(v_sb[:, kc, :], vt[:])

        # ---- main loop over row-tiled key-chunk pairs ----
        with tc.tile_pool(name="s_ps", bufs=2, space="PSUM") as s_ps_p, tc.tile_pool(
            name="o_ps", bufs=1, space="PSUM"
        ) as o_ps_p, tc.tile_pool(name="pt", bufs=12) as pt_p, tc.tile_pool(
            name="zz", bufs=8
        ) as z_p, tc.tile_pool(name="v2", bufs=4) as v2_p:
            out_ps = o_ps_p.tile([128, HQ], f32)
            # Zero-init every out bank with a full-partition start=True matmul
            # so the packed q-halves can then both accumulate with start=False
            # (avoids interleaved-group hazards on the bank-wide has_written
            # clear); a matching full-partition stop=True zero-add closes them.
            zw = const_p.tile([128, max(BW, 128)], bf16)
            nc.vector.memset(zw[:], 0.0)
            for j in range(HB):
                nc.tensor.matmul(
                    out_ps[:, j * BW : (j + 1) * BW],
                    lhsT=zw[:, 0:128],
                    rhs=zw[:, 0:BW],
                    start=True,
                    stop=False,
                )

            def emit_out_mms(kc, pts, v2):
                # pair the q-halves adjacently so the (0,0)/(0,64) col-tiled
                # matmuls run concurrently in the array
                for i in range(HB):
                    for half in range(2):
                        q0 = half * HQ + i * BW
                        w, j = q0 // WW, (q0 % WW) // BW
                        nc.tensor.matmul(
                            out_ps[half * 64 : (half + 1) * 64, i * BW : (i + 1) * BW],
                            lhsT=v2[:],
                            rhs=pts[w][:, j * BW : (j + 1) * BW],
                            start=False,
                            stop=False,
                        )

            def finish_kc(kc, zacc):
                """Z -> 1/Z -> V' for one key chunk."""
                z = z_p.tile([128, 1], f32, tag="z")
                nc.vector.reduce_sum(z[:], zacc[:], axis=mybir.AxisListType.X)
                rz = z_p.tile([128, 1], f32, tag="rz")
                nc.vector.reciprocal(rz[:], z[:])
                v2 = v2_p.tile([128, H], bf16, tag="v2")
                nc.vector.tensor_scalar_mul(v2[:], v_sb[:, kc, :], rz[:])
                return v2

            def emit_pair(kcA, kcB):
                """Score windows + exp + Z + V' for a row-tiled key-chunk
                pair: kcA's matmuls use array rows 0-63, kcB's rows 64-127,
                emitted adjacently so the hardware runs each A/B pair
                concurrently."""
                kslA = kk_sb[0:64, kcA * 128 : (kcA + 1) * 128]
                kslB = kk_sb[64:128, kcB * 128 : (kcB + 1) * 128]
                zaccA = z_p.tile([128, NW], f32, tag="zaccA")
                zaccB = z_p.tile([128, NW], f32, tag="zaccB")
                ptsA, ptsB = [], []
                for w in range(NW):
                    sA = s_ps_p.tile([128, WW], f32, tag="s")
                    sB = s_ps_p.tile([128, WW], f32, tag="s")
                    for j in range(NB):
                        q0 = w * WW + j * BW
                        nc.tensor.matmul(
                            sA[:, j * BW : (j + 1) * BW],
                            lhsT=kslA,
                            rhs=qq_sb[0:64, q0 : q0 + BW],
                            start=True,
                            stop=True,
                        )
                        nc.tensor.matmul(
                            sB[:, j * BW : (j + 1) * BW],
                            lhsT=kslB,
                            rhs=qq_sb[64:128, q0 : q0 + BW],
                            start=True,
                            stop=True,
                        )
                    for s_ps, pts, zacc in (
                        (sA, ptsA, zaccA),
                        (sB, ptsB, zaccB),
                    ):
                        pt = pt_p.tile([128, WW], bf16, tag="pt")
                        pts.append(pt)
                        if w < NW // 2 or NW == 1:
                            nc.scalar.activation(
                                pt[:], s_ps[:], AF.Exp, scale=scale,
                                accum_out=zacc[:, w : w + 1],
                            )
                        else:
                            nc.scalar.activation(pt[:], s_ps[:], AF.Exp, scale=scale)
                            nc.vector.reduce_sum(
                                zacc[:, w : w + 1], pt[:], axis=mybir.AxisListType.X
                            )
                return (ptsA, finish_kc(kcA, zaccA)), (ptsB, finish_kc(kcB, zaccB))

            prev = []
            for kp in range(KC // 2):
                curA, curB = emit_pair(2 * kp, 2 * kp + 1)
                for p in prev:
                    emit_out_mms(0, *p)
                prev = [curA, curB]
            for p in prev:
                emit_out_mms(0, *p)

            # close every accumulation group with a full-partition zero-add
            for j in range(HB):
                nc.tensor.matmul(
                    out_ps[:, j * BW : (j + 1) * BW],
                    lhsT=zw[:, 0:128],
                    rhs=zw[:, 0:BW],
                    start=False,
                    stop=True,
                )
            out_sb = const_p.tile([128, HQ], f32)
            nc.vector.tensor_copy(out_sb[:], out_ps[:])
            nc.sync.dma_start(out=out_d.ap(), in_=out_sb[:])

    nc.compile()
    return nc


_NC_CACHE = {}


def _get_nc(seq=SEQ):
    if seq not in _NC_CACHE:
        _NC_CACHE[seq] = build_graph(seq)
    return _NC_CACHE[seq]


def make_in_maps(input_ids, Wq, Wk):
    bf = ml_dtypes.bfloat16
    wq_t = np.ascontiguousarray(Wq.T).astype(bf)  # [E, H]
    wk_t = np.ascontiguousarray(Wk.T).astype(bf)
    wq2 = np.concatenate([wq_t, wq_t], axis=1)  # [E, 128]
    wk2 = np.concatenate([wk_t, wk_t], axis=1)
    in_maps = []
    for b in range(input_ids.shape[0]):
        xt = np.ascontiguousarray(input_ids[b].T).astype(bf)  # [E, S]
        in_maps.append({"xt": xt, "wq2": wq2, "wk2": wk2})
    return in_maps


def unpack_out(packed, seq=SEQ):
    """[128, seq/2] packed out^T -> [seq, H] out."""
    ot = np.concatenate([packed[:64], packed[64:]], axis=1)  # [64, seq]
    return np.ascontiguousarray(ot.T)


def run(input_ids, Wq, Wk, trace=False, **kwargs):
    from concourse.bass_utils import run_bass_kernel_spmd

    nc = _get_nc(input_ids.shape[1])
    in_maps = make_in_maps(input_ids, Wq, Wk)
    res = run_bass_kernel_spmd(
        nc, in_maps, core_ids=list(range(len(in_maps))), trace=trace, **kwargs
    )
    out = np.stack(
        [
            unpack_out(res.results[b]["out"], input_ids.shape[1])
            for b in range(len(in_maps))
        ]
    ).astype(np.float32)
    return out, res


def kernel(input_ids, Wq, Wk, Wv):
    out, _ = run(np.asarray(input_ids), np.asarray(Wq), np.asarray(Wk))
    return out
